# revision 65
# baseline (speedup 1.0000x reference)
"""Multi-head attention (B=4, S=2048, M=1024, H=16, D=64) on 8 trn2 cores.

Sharding: core c = (b, g) with b = c // 2 (batch), g = c % 2 (head group of 8
heads).  Each core computes q/k/v projections for its 8 heads, causal
attention, and a partial output projection (contraction over its 512 feature
rows of Wo).  Host sums the two partials per batch and adds the bias.

Perf design (613us baseline -> 278us):
  * q/k projections in fp8e4m3 DoubleRow (2 contraction values per PE row;
    host interleaves m = 256*mc + 2*p + i and scales wq/wk by 16, the exp
    scale divides the 256x back out); v / scores / PV / out-proj in bf16.
  * Weights loaded once; x loaded per s-block with host-pretiled layouts so
    DMA rows are contiguous per partition (split in 4 chunks per block: one
    1MB burst measurably slows concurrent PE instruction fetch).
  * Causal tri mask added on the PE itself (identity-stationary matmul
    accumulated into the open score PSUM group) - keeps the DVE off the
    score->exp critical path.
  * Scores for a head pair packed into one [128, 1024] two-bank PSUM tile
    so exp() runs as a single wide ACT instruction (strided 3D AP covers
    both heads' live regions on diagonal chunks).
  * Projection / output-projection matmul groups interleaved into the
    attention chunk stream as PE fillers, spaced to last the whole block.
    A gap-free PE stream keeps the clock at the 2.4GHz p-state (idle gaps
    drop it to 1.2/0.65GHz, which is what made the baseline 2x slower).
  * Softmax denominator rides as a ones-column in v ([v_h|1]) so the PV
    matmul accumulates it for free; normalization uses
    reciprocal_approx_fast + an ind8-stationary broadcast matmul.

Device-side layouts:
  xqT/xkT/xvT  tiled [j*128, mc*512]  host-transposed activations
  qT,kT        [128, 2048] x4       2 heads per tile (rows 0:64 / 64:128)
  v            [128, 520] x16       per 128-row tile: cols 65h..65h+63 = v_h,
                                    col 65h+64 = 1.0 (softmax denominator)
  sc           [128(sk), 1024] f32 PSUM: cols 0:512 head A, 512:1024 head B
  pt           exp((sc + mask)*scl) bf16, no max subtraction (|scores/8| small)
  pv_h         [65, 512] f32 PSUM   = [v_h|1].T @ pt_h ; row 64 = denom
  at           [128, 512]           pair attn rows / denom, bf16
  out_partial  [2048, 1024] bf16    = attnT.T @ wo   (no bias)
"""

import os
import sys

for _p in ("/opt/trn_rl_repo", "/root/.axon_site/_ro/trn_rl_repo"):
    if os.path.isdir(_p) and _p not in sys.path:
        sys.path.append(_p)

import numpy as np
import ml_dtypes

B, S, M, H, D = 4, 2048, 1024, 16, 64
G = 2               # head groups (cores per batch)
HPG = H // G        # heads per group = 8
DH = HPG * D        # feature rows per group = 512
NCORES = B * G
SB = 512            # sq block (matmul N)
CK = 128            # sk chunk (matmul M / partition)
NJ = S // SB        # 4 sq blocks
NC = S // CK        # 16 sk chunks
MK = M // 128       # 8 m chunks

_PROG_CACHE = {}


def _build_program(variant):
    """variant: 'causal' | 'allones' | 'general'"""
    import concourse.bass as bass
    import concourse.bacc as bacc
    import concourse.mybir as mybir
    from concourse import tile
    from contextlib import ExitStack

    f32 = mybir.dt.float32
    bf16 = mybir.dt.bfloat16
    nc = bacc.Bacc("TRN2", target_bir_lowering=False, debug=False, num_devices=NCORES)

    # host pre-tiles x as [j, p, mc, s] and w as [p, mc, d] so every DMA
    # reads DRAM rows that are contiguous per SBUF partition; the device
    # splits each block load into 4 column-chunks so transfers interleave
    # with other traffic instead of monopolizing HBM in 1MB bursts
    # q/k run the projections in fp8e4m3 DoubleRow (contraction pairs
    # interleaved: m = 256*mc + 2*p + i -> row (j, p), col (mc, i, s));
    # host scales wq/wk by 16 so small weights stay in fp8 normal range,
    # and the exp() scale divides the resulting 256x score factor back out.
    fp8 = mybir.dt.float8e4
    xqT = nc.dram_tensor("xqT", [NJ * 128, MK * SB], fp8, kind="ExternalInput").ap()
    xkT = nc.dram_tensor("xkT", [NJ * 128, MK * SB], fp8, kind="ExternalInput").ap()
    xvT = nc.dram_tensor("xvT", [NJ * 128, MK * SB], bf16, kind="ExternalInput").ap()
    wq = nc.dram_tensor("wq", [128, MK * DH], fp8, kind="ExternalInput").ap()
    wk = nc.dram_tensor("wk", [128, MK * DH], fp8, kind="ExternalInput").ap()
    wv = nc.dram_tensor("wv", [128, MK * DH], bf16, kind="ExternalInput").ap()
    wo = nc.dram_tensor("wo", [128, 4 * M], bf16, kind="ExternalInput").ap()
    tri = nc.dram_tensor("tri", [128, 256], bf16, kind="ExternalInput").ap()
    eye = nc.dram_tensor("eye", [128, 128], bf16, kind="ExternalInput").ap()
    ind8 = nc.dram_tensor("ind8", [8, SB], bf16, kind="ExternalInput").ap()
    if variant == "general":
        maskT = nc.dram_tensor("maskT", [S, S], bf16, kind="ExternalInput").ap()
    out = nc.dram_tensor("out", [S, M], bf16, kind="ExternalOutput").ap()

    with tile.TileContext(nc) as tc, ExitStack() as ctx:
        ep = ctx.enter_context
        ctx.enter_context(nc.allow_low_precision(reason="bf16 matmul inputs"))
        dma = nc.sync.dma_start

        w_pool = ep(tc.tile_pool(name="w", bufs=1))
        x_pool = ep(tc.tile_pool(name="x", bufs=6))
        qT_pool = ep(tc.tile_pool(name="qT", bufs=1))
        kT_pool = ep(tc.tile_pool(name="kT", bufs=1))
        v_pool = ep(tc.tile_pool(name="v", bufs=1))
        pt_pool = ep(tc.tile_pool(name="pt", bufs=4))
        at_pool = ep(tc.tile_pool(name="at", bufs=12))
        nrm_pool = ep(tc.tile_pool(name="nrm", bufs=2))
        out_pool = ep(tc.tile_pool(name="outp", bufs=4))
        misc_pool = ep(tc.tile_pool(name="misc", bufs=1))
        if variant == "general":
            mk_pool = ep(tc.tile_pool(name="mk", bufs=4))

        ps_sc = ep(tc.tile_pool(name="ps_sc", bufs=2, space="PSUM"))
        ps_pv = ep(tc.tile_pool(name="ps_pv", bufs=2, space="PSUM"))
        ps_mm = ep(tc.tile_pool(name="ps_mm", bufs=2, space="PSUM"))

        # ---- x loads: one big DMA per (kind, s-block) ----
        x_dram_map = {"q": xqT, "k": xkT, "v": xvT}
        x_tiles = {}          # (kind, j) -> [128, MK*SB] tile, mc-major

        def load_x(kind, j):
            x_dram = x_dram_map[kind]
            dt = bf16 if kind == "v" else fp8
            xt = x_pool.tile([128, MK * SB], dt, name=f"x_{kind}{j}",
                             tag="xv" if kind == "v" else "x8",
                             bufs=2 if kind == "v" else 4)
            q = MK * SB // 4
            for h in range(4):
                dma(xt[:, h * q:(h + 1) * q],
                    x_dram[j * 128:(j + 1) * 128, h * q:(h + 1) * q])
            x_tiles[(kind, j)] = xt

        def x_sl(kind, j, mc):
            return x_tiles[(kind, j)][:, mc * SB:(mc + 1) * SB]

        # ---- prologue: interleave x block 0 (sync queue) w/ weights
        # (gpsimd queue) so the first projection group starts ASAP ----
        w_sb = {}

        def load_w(nm, w_dram):
            dt = bf16 if nm == "v" else fp8
            wt = w_pool.tile([128, MK * DH], dt, name=f"w_{nm}")
            q = MK * DH // 4
            for h in range(4):
                nc.gpsimd.dma_start(wt[:, h * q:(h + 1) * q],
                                    w_dram[:, h * q:(h + 1) * q])
            w_sb[nm] = wt

        load_x("q", 0)
        load_w("q", wq)
        load_x("k", 0)
        load_w("k", wk)
        load_x("v", 0)
        load_w("v", wv)

        tri_sb = misc_pool.tile([128, 256], bf16, name="tri_sb")
        dma(tri_sb[:], tri[:])
        eye_sb = misc_pool.tile([128, 128], bf16, name="eye_sb")
        dma(eye_sb[:], eye[:])
        ind8_sb = misc_pool.tile([8, SB], bf16, name="ind8_sb")
        dma(ind8_sb[:], ind8[:])

        wo_sb = w_pool.tile([128, 4 * M], bf16, name="w_o")
        for h in range(4):
            nc.gpsimd.dma_start(wo_sb[:, h * M:(h + 1) * M],
                                wo[:, h * M:(h + 1) * M])

        qT_sb = [qT_pool.tile([128, S], bf16, name=f"qT{d}") for d in range(4)]
        kT_sb = [kT_pool.tile([128, S], bf16, name=f"kT{d}") for d in range(4)]
        v_sb = [v_pool.tile([128, HPG * 65], bf16, name=f"v{t}") for t in range(NC)]

        for t in range(NC):
            v3 = v_sb[t].rearrange("p (h c) -> p h c", h=HPG, c=65)
            nc.gpsimd.memset(v3[:, :, 64:65], 1.0)

        def load_x_block(j):
            for kind in ("q", "k", "v"):
                load_x(kind, j)

        # ---- projection groups (emitted directly or as fillers) ----
        def proj_group_qk(kind, j, d):
            def emit():
                ps = ps_mm.tile([128, SB], f32, name=f"pj_{kind}{j}_{d}",
                                tag="mm")
                w4 = w_sb[kind].rearrange("p (c i d) -> p c i d",
                                          c=MK // 2, i=2, d=DH)
                x4 = x_tiles[(kind, j)].rearrange("p (c i s) -> p c i s",
                                                  c=MK // 2, i=2, s=SB)
                for mc in range(MK // 2):
                    nc.tensor.matmul(
                        ps[:],
                        w4[:, mc, :, d * 128:(d + 1) * 128],
                        x4[:, mc, :, :],
                        start=(mc == 0), stop=(mc == MK // 2 - 1),
                        perf_mode=mybir.MatmulPerfMode.DoubleRow)
                dst = qT_sb if kind == "q" else kT_sb
                nc.vector.tensor_copy(dst[d][:, j * SB:(j + 1) * SB], ps[:])
            return emit

        def proj_group_v(j, st):
            def emit():
                t = 4 * j + st
                ps = ps_mm.tile([128, DH], f32, name=f"pj_v{t}", tag="mm")
                w = w_sb["v"]
                for mc in range(MK):
                    nc.tensor.matmul(
                        ps[:],
                        x_sl("v", j, mc)[:, st * 128:(st + 1) * 128],
                        w[:, mc * DH:(mc + 1) * DH],
                        start=(mc == 0), stop=(mc == MK - 1))
                v3 = v_sb[t].rearrange("p (h c) -> p h c", h=HPG, c=65)
                p3 = ps.rearrange("p (h c) -> p h c", h=HPG, c=64)
                nc.vector.tensor_copy(v3[:, :, 0:64], p3[:])
            return emit

        def proj_block_groups(j):
            gs = []
            for kind in ("q", "k"):
                for d in range(4):
                    gs.append(proj_group_qk(kind, j, d))
            for st in range(4):
                gs.append(proj_group_v(j, st))
            return gs

        # ---- output projection groups for block j (need at_tiles[j]) ----
        at_tiles = {}         # j -> list of 4 pair tiles

        def outproj_groups(j):
            gs = []
            for ss in range(4):
                for nh in range(2):
                    def emit(ss=ss, nh=nh):
                        ps = ps_mm.tile([128, SB], f32, name=f"po{j}_{ss}_{nh}",
                                        tag="mm")
                        for d in range(4):
                            nc.tensor.matmul(
                                ps[:],
                                at_tiles[j][d][:, ss * 128:(ss + 1) * 128],
                                wo_sb[:, d * M + nh * SB: d * M + nh * SB + SB],
                                start=(d == 0), stop=(d == 3))
                        ot = out_pool.tile([128, SB], bf16,
                                           name=f"ot{j}_{ss}_{nh}", tag="ot")
                        # final block: split casts/stores across engines so
                        # the epilogue drains in parallel
                        if j == NJ - 1 and (ss + nh) % 2 == 1:
                            nc.scalar.copy(ot[:], ps[:])
                        else:
                            nc.vector.tensor_copy(ot[:], ps[:])
                        r0 = j * SB + ss * 128
                        dst = out[r0:r0 + 128, nh * SB:(nh + 1) * SB]
                        if j == NJ - 1 and ss % 2 == 1:
                            dma(dst, ot[:])
                        else:
                            nc.gpsimd.dma_start(dst, ot[:])
                    gs.append(emit)
            return gs

        # ---- attention block ----
        Exp = mybir.ActivationFunctionType.Exp
        SCL = 0.125 / 256.0   # 1/sqrt(64), divided by the fp8 16x16 scaling

        def attn_block(j, fillers):
            nchunks = 4 * (j + 1) if variant == "causal" else NC
            # space fillers so the supply lasts through the whole block
            # (but at most one every 2 chunks to avoid crowding out scores)
            total_slots = (HPG // 2) * nchunks
            itv = max(2, total_slots // (len(fillers) + 1)) if fillers else 0
            slot = 0
            fi = 0
            ats = []
            dn_all = nrm_pool.tile([8, SB], f32, name=f"dn{j}", tag="dn")
            for hp in range(HPG // 2):
                hA, hB = 2 * hp, 2 * hp + 1
                pvA = ps_pv.tile([65, SB], f32, name=f"pv{j}_{hA}", tag="pv")
                pvB = ps_pv.tile([65, SB], f32, name=f"pv{j}_{hB}", tag="pv")
                for c in range(nchunks):
                    diag = variant == "causal" and c >= 4 * j
                    o = 128 * (c - 4 * j) if diag else 0
                    sc = ps_sc.tile([128, 2 * SB], f32, name=f"sc{j}_{hp}_{c}",
                                    tag="sc")
                    for side, h in ((0, hA), (1, hB)):
                        drow = 64 * side
                        reg = side * SB + o
                        nc.tensor.matmul(
                            sc[:, reg:side * SB + SB],
                            kT_sb[hp][drow:drow + 64, c * CK:(c + 1) * CK],
                            qT_sb[hp][drow:drow + 64,
                                      j * SB + o:(j + 1) * SB],
                            start=True, stop=not diag)
                    if diag:
                        for side in (0, 1):
                            reg = side * SB + o
                            nc.tensor.matmul(
                                sc[:, reg:reg + 128], eye_sb[:],
                                tri_sb[:, 0:128], start=False, stop=True)
                    if variant == "general":
                        mk = mk_pool.tile([128, SB], bf16,
                                          name=f"mk{j}_{hp}_{c}", tag="mk")
                        nc.gpsimd.dma_start(
                            mk[:], maskT[c * CK:(c + 1) * CK,
                                         j * SB:(j + 1) * SB])
                        nc.vector.tensor_add(sc[:, 0:SB], sc[:, 0:SB], mk[:])
                        nc.vector.tensor_add(sc[:, SB:2 * SB],
                                             sc[:, SB:2 * SB], mk[:])
                    pt = pt_pool.tile([128, 2 * SB], bf16,
                                      name=f"pt{j}_{hp}_{c}", tag="pt")
                    if diag and o > 0:
                        # one strided activation covers both heads' live
                        # [o:SB] regions
                        sc3 = sc.rearrange("p (g c) -> p g c", g=2, c=SB)
                        pt3 = pt.rearrange("p (g c) -> p g c", g=2, c=SB)
                        nc.scalar.activation(pt3[:, :, o:SB], sc3[:, :, o:SB],
                                             Exp, scale=SCL)
                    else:
                        nc.scalar.activation(pt[:, 0:2 * SB], sc[:, 0:2 * SB],
                                             Exp, scale=SCL)
                    nc.tensor.matmul(
                        pvA[:, o:SB], v_sb[c][:, 65 * hA:65 * hA + 65],
                        pt[:, o:SB],
                        start=(c == 0), stop=(c == nchunks - 1))
                    nc.tensor.matmul(
                        pvB[:, o:SB], v_sb[c][:, 65 * hB:65 * hB + 65],
                        pt[:, SB + o:2 * SB],
                        start=(c == 0), stop=(c == nchunks - 1))
                    slot += 1
                    if itv and slot % itv == 0 and fi < len(fillers):
                        fillers[fi]()
                        fi += 1
                # drain the pair
                at = at_pool.tile([128, SB], bf16, name=f"at{j}_{hp}",
                                  tag="at")
                ats.append(at)
                for h, pv, drow in ((hA, pvA, 0), (hB, pvB, 64)):
                    nc.vector.tensor_copy(at[drow:drow + 64, :], pv[0:64, :])
                    dnt = nrm_pool.tile([1, SB], f32, name=f"dnt{j}_{h}",
                                        tag="dnt")
                    nc.vector.tensor_copy(dnt[:], pv[64:65, :])
                    dma(dn_all[h:h + 1, :], dnt[:])
            # flush leftover fillers first so the PE chews on them while the
            # DVE/DMA denominator chain completes
            while fi < len(fillers):
                fillers[fi]()
                fi += 1
            # normalization: at /= denom (broadcast via ind8 matmul)
            dnr = nrm_pool.tile([8, SB], f32, name=f"dnr{j}", tag="dnr",
                                bufs=2)
            nc.vector.reciprocal_approx_fast(dnr[:], dn_all[:])
            dnr_bf = nrm_pool.tile([8, SB], bf16, name=f"dnrb{j}", tag="dnrb",
                                   bufs=2)
            nc.vector.tensor_copy(dnr_bf[:], dnr[:])
            for hp in range(HPG // 2):
                rb = ps_mm.tile([128, SB], f32, name=f"rb{j}_{hp}", tag="mm")
                nc.tensor.matmul(
                    rb[:], ind8_sb[:, hp * 128:(hp + 1) * 128], dnr_bf[:],
                    start=True, stop=True)
                nc.vector.tensor_mul(ats[hp][:], ats[hp][:], rb[:])
            at_tiles[j] = ats

        # ---- schedule ---- (x block 0 already loading in the prologue)
        load_x_block(1)
        for g in proj_block_groups(0):
            g()
        for j in range(NJ):
            if j + 2 < NJ:
                load_x_block(j + 2)
            fillers = []
            if j + 1 < NJ:
                fillers += proj_block_groups(j + 1)
            if j >= 1:
                fillers += outproj_groups(j - 1)
            attn_block(j, fillers)
        for g in outproj_groups(NJ - 1):
            g()

    nc.compile()
    return nc


def _get_program(variant):
    if variant not in _PROG_CACHE:
        _PROG_CACHE[variant] = _build_program(variant)
    return _PROG_CACHE[variant]


def _to_bf16(a):
    return np.asarray(a, np.float32).astype(ml_dtypes.bfloat16)


def _to_fp8(a):
    return np.asarray(a, np.float32).astype(ml_dtypes.float8_e4m3)


def _host_prep(queries, keys, values, masks, Wq, Wk, Wv):
    """Build the 8 per-core input maps."""
    tril = np.tril(np.ones((S, S), dtype=bool))
    if all(np.array_equal(masks[b], tril) for b in range(B)):
        variant = "causal"
    elif masks.all():
        variant = "allones"
    else:
        variant = "general"

    sq = np.arange(128)
    tri_np = np.where(sq[None, :] >= sq[:, None], 0.0, -1.0e6).astype(np.float32)
    tri_np = np.concatenate([tri_np, tri_np], axis=1)   # both heads' regions
    eye_np = np.eye(128, dtype=np.float32)
    ind8_np = np.zeros((8, 512), np.float32)
    for c in range(4):
        for cc in range(128):
            ind8_np[2 * c + cc // 64, 128 * c + cc] = 1.0

    # [H, M, D] -> [M, H*D] head-major per group, then m-chunk tiled so the
    # sbuf row for partition p is contiguous in DRAM: [p, mc, d] (bf16) or
    # DoubleRow-interleaved [p, mc, i, d] with m = 256*mc + 2*p + i (fp8)
    def wcat(w, g):
        wm = w[g * HPG:(g + 1) * HPG].transpose(1, 0, 2).reshape(M, DH)
        return _to_bf16(np.ascontiguousarray(
            wm.reshape(MK, 128, DH).transpose(1, 0, 2).reshape(128, MK * DH)))

    def wcat8(w, g):
        wm = w[g * HPG:(g + 1) * HPG].transpose(1, 0, 2).reshape(M, DH)
        wm = wm.reshape(MK // 2, 128, 2, DH).transpose(1, 0, 2, 3)
        return _to_fp8(np.ascontiguousarray(
            wm.reshape(128, MK * DH)) * 16.0)

    # x [S, M] -> xT tiled [j, p, mc, s] (bf16) / [j, p, mc, i, s] (fp8)
    def xtile(x):
        xT = x.T.reshape(MK, 128, NJ, SB).transpose(2, 1, 0, 3)
        return _to_bf16(np.ascontiguousarray(
            xT.reshape(NJ * 128, MK * SB)))

    def xtile8(x):
        xT = x.T.reshape(MK // 2, 128, 2, NJ, SB).transpose(3, 1, 0, 2, 4)
        return _to_fp8(np.ascontiguousarray(
            xT.reshape(NJ * 128, MK * SB)))

    xq = [xtile8(queries[b]) for b in range(B)]
    xk = [xtile8(keys[b]) for b in range(B)]
    xv = [xtile(values[b]) for b in range(B)]
    wq_g = [wcat8(Wq, g) for g in range(G)]
    wk_g = [wcat8(Wk, g) for g in range(G)]
    wv_g = [wcat(Wv, g) for g in range(G)]
    tri_bf = _to_bf16(tri_np)
    eye_bf = _to_bf16(eye_np)
    ind8_bf = _to_bf16(ind8_np)
    if variant == "general":
        mask_bf = [_to_bf16(np.where(masks[b].T, 0.0, -1.0e6).astype(np.float32))
                   for b in range(B)]

    in_maps = []
    for c in range(NCORES):
        b, g = c // G, c % G
        m = {
            "xqT": xq[b],
            "xkT": xk[b],
            "xvT": xv[b],
            "wq": wq_g[g],
            "wk": wk_g[g],
            "wv": wv_g[g],
            "tri": tri_bf,
            "eye": eye_bf,
            "ind8": ind8_bf,
        }
        if variant == "general":
            m["maskT"] = mask_bf[b]
        in_maps.append(m)
    return variant, in_maps


def run(queries, keys, values, masks, Wq, Wk, Wv, Wo, bo, trace=False):
    from concourse import bass_utils

    queries = np.asarray(queries, np.float32)
    keys = np.asarray(keys, np.float32)
    values = np.asarray(values, np.float32)
    masks = np.asarray(masks, bool)
    Wq = np.asarray(Wq, np.float32)
    Wk = np.asarray(Wk, np.float32)
    Wv = np.asarray(Wv, np.float32)
    Wo = np.asarray(Wo, np.float32)
    bo = np.asarray(bo, np.float32)

    variant, in_maps = _host_prep(queries, keys, values, masks, Wq, Wk, Wv)
    wo_t = []
    for g in range(G):
        w = Wo[g * DH:(g + 1) * DH, :]
        wo_t.append(_to_bf16(np.ascontiguousarray(
            w.reshape(4, 128, M).transpose(1, 0, 2).reshape(128, 4 * M))))
    for c in range(NCORES):
        in_maps[c]["wo"] = wo_t[c % G]

    nc = _get_program(variant)
    res = bass_utils.run_bass_kernel_spmd(
        nc, in_maps, list(range(NCORES)), trace=trace)

    out = np.empty((B, S, M), np.float32)
    for b in range(B):
        out[b] = (res.results[G * b]["out"].astype(np.float32)
                  + res.results[G * b + 1]["out"].astype(np.float32) + bo)
    return out, res


def kernel(queries, keys, values, masks, Wq, Wk, Wv, Wo, bo):
    out, _ = run(queries, keys, values, masks, Wq, Wk, Wv, Wo, bo, trace=False)
    return out
